# revision 8
# baseline (speedup 1.0000x reference)
"""MSE AutogradLoss kernel for 8 TRN2 NeuronCores.

loss      = (x - y)^2                  [B, 16]
d_loss    = 2 (x - y)                  [B, 16]
sqd_loss  = per-sample Hessian = 2*I   [B, 16, 16]  (input-independent constant)

Pure data parallel: batch 1048576 split 8 ways (131072 rows/core).
Per core the kernel is HBM-bound: 16 MiB reads + 144 MiB writes.
The Hessian output is generated on-chip (memset pattern in SBUF) and
streamed to DRAM with large repeated DMAs - no HBM reads for it.
"""

import numpy as np

import concourse.bacc as bacc
import concourse.bass as bass
import concourse.mybir as mybir
from concourse.bass_utils import run_bass_kernel_spmd
from concourse.tile import TileContext

F32 = mybir.dt.float32

B = 1048576
D = 16
N_CORES = 8
B_SHARD = B // N_CORES  # 131072
P = 128


def build_nc(b_shard: int = B_SHARD, chunk_cols: int = 4096, hess_k: int = 64):
    """Build the per-core Bass graph.

    Layouts (per core, partition-major flattening of the batch):
      x/y/loss/dloss: [128, b_shard*16/128]   partition p holds batch rows
                                              [p*b_shard/128, (p+1)*b_shard/128)
      sqd:            [128, b_shard*256/128]  same row ownership
    """
    free = b_shard * D // P
    hfree = b_shard * D * D // P
    hess_cols = hess_k * 256
    assert free % chunk_cols == 0
    assert hfree % hess_cols == 0

    nc = bacc.Bacc()
    x = nc.declare_dram_parameter("x", [P, free], F32, isOutput=False)
    y = nc.declare_dram_parameter("y", [P, free], F32, isOutput=False)
    loss = nc.declare_dram_parameter("loss", [P, free], F32, isOutput=True)
    dloss = nc.declare_dram_parameter("dloss", [P, free], F32, isOutput=True)
    sqd = nc.declare_dram_parameter("sqd", [P, hfree], F32, isOutput=True)

    with TileContext(nc) as tc:
        with (
            tc.tile_pool(name="hpool", bufs=1) as hpool,
            tc.tile_pool(name="io", bufs=3) as io,
        ):
            # Constant Hessian pattern: per partition, hess_k copies of the
            # flattened 2*eye(16) (256 elements, 2.0 at offsets 0,17,..,255).
            hess = hpool.tile([P, hess_cols], F32)
            nc.vector.memset(hess[:, :], 0.0)
            hv = hess.rearrange("p (k c) -> p k c", c=256)
            for i in range(D):
                nc.vector.memset(hv[:, :, 17 * i], 2.0)

            # Stream the constant tile to the full Hessian output.
            # Dedicated sync-engine HWDGE ring: after the fill these DMAs
            # have no waits, so nothing head-of-line blocks the stream.
            for j in range(hfree // hess_cols):
                nc.sync.dma_start(
                    out=sqd[:, j * hess_cols : (j + 1) * hess_cols], in_=hess[:, :]
                )

            # loss / d_loss pipeline. Loads on the gpsimd SWDGE ring and
            # stores on the scalar HWDGE ring so the three DMA streams
            # dispatch independently. DVE computes in place:
            #   xt: x -> d -> 2d (dloss),  yt: y -> d^2 (loss)
            for c in range(free // chunk_cols):
                sl = bass.ts(c, chunk_cols)
                xt = io.tile([P, chunk_cols], F32, tag="xt")
                yt = io.tile([P, chunk_cols], F32, tag="yt")
                nc.sync.dma_start(out=xt[:, :], in_=x[:, sl])
                nc.sync.dma_start(out=yt[:, :], in_=y[:, sl])
                nc.vector.tensor_sub(xt[:, :], xt[:, :], yt[:, :])
                nc.vector.tensor_mul(yt[:, :], xt[:, :], xt[:, :])
                nc.vector.tensor_add(xt[:, :], xt[:, :], xt[:, :])
                nc.sync.dma_start(out=loss[:, sl], in_=yt[:, :])
                nc.sync.dma_start(out=dloss[:, sl], in_=xt[:, :])
    nc.finalize()
    return nc


TRACE = False  # test.py sets True to capture exec_time_ns
last_results = None


def kernel(model_out: np.ndarray, y_true: np.ndarray):
    global last_results
    model_out = np.ascontiguousarray(model_out, dtype=np.float32)
    y_true = np.ascontiguousarray(y_true, dtype=np.float32)
    free = B_SHARD * D // P

    nc = build_nc()
    in_maps = []
    for c in range(N_CORES):
        rows = slice(c * B_SHARD, (c + 1) * B_SHARD)
        in_maps.append(
            {
                "x": model_out[rows].reshape(P, free),
                "y": y_true[rows].reshape(P, free),
            }
        )
    res = run_bass_kernel_spmd(nc, in_maps, core_ids=list(range(N_CORES)), trace=TRACE)
    last_results = res

    loss = np.concatenate([r["loss"].reshape(B_SHARD, D) for r in res.results])
    dloss = np.concatenate([r["dloss"].reshape(B_SHARD, D) for r in res.results])
    sqd = np.concatenate([r["sqd"].reshape(B_SHARD, D, D) for r in res.results])
    return loss, dloss, sqd


# revision 10
# speedup vs baseline: 1.2473x; 1.2473x over previous
"""MSE AutogradLoss kernel for 8 TRN2 NeuronCores.

loss      = (x - y)^2                  [B, 16]
d_loss    = 2 (x - y)                  [B, 16]
sqd_loss  = per-sample Hessian = 2*I   [B, 16, 16]  (input-independent constant)

Pure data parallel: batch 1048576 split 8 ways (131072 rows/core).
Per core the kernel is HBM-bound: 16 MiB reads + 144 MiB writes.
The Hessian output is generated on-chip (memset pattern in SBUF) and
streamed to DRAM with large repeated DMAs - no HBM reads for it.
"""

import numpy as np

import concourse.bacc as bacc
import concourse.bass as bass
import concourse.mybir as mybir
from concourse.bass_utils import run_bass_kernel_spmd
from concourse.tile import TileContext

F32 = mybir.dt.float32

B = 1048576
D = 16
N_CORES = 8
B_SHARD = B // N_CORES  # 131072
P = 128


def build_nc(b_shard: int = B_SHARD, chunk_cols: int = 2048, hess_k: int = 32):
    """Build the per-core Bass graph.

    Layouts (per core, partition-major flattening of the batch):
      x/y/loss/dloss: [128, b_shard*16/128]   partition p holds batch rows
                                              [p*b_shard/128, (p+1)*b_shard/128)
      sqd:            [128, b_shard*256/128]  same row ownership
    """
    free = b_shard * D // P
    hfree = b_shard * D * D // P
    hess_cols = hess_k * 256
    assert free % chunk_cols == 0
    assert hfree % hess_cols == 0

    nc = bacc.Bacc()
    x = nc.declare_dram_parameter("x", [P, free], F32, isOutput=False)
    y = nc.declare_dram_parameter("y", [P, free], F32, isOutput=False)
    loss = nc.declare_dram_parameter("loss", [P, free], F32, isOutput=True)
    dloss = nc.declare_dram_parameter("dloss", [P, free], F32, isOutput=True)
    sqd = nc.declare_dram_parameter("sqd", [P, hfree], F32, isOutput=True)

    with TileContext(nc) as tc:
        with (
            tc.tile_pool(name="hpool", bufs=1) as hpool,
            tc.tile_pool(name="io", bufs=3) as io,
        ):
            # Constant Hessian pattern: per partition, hess_k copies of the
            # flattened 2*eye(16) (256 elements, 2.0 at offsets 0,17,..,255).
            hess = hpool.tile([P, hess_cols], F32)
            nc.vector.memset(hess[:, :], 0.0)
            hv = hess.rearrange("p (k c) -> p k c", c=256)
            for i in range(D):
                nc.vector.memset(hv[:, :, 17 * i], 2.0)

            # Stream the constant tile to the full Hessian output.
            # Dedicated sync-engine HWDGE ring: after the fill these DMAs
            # have no waits, so nothing head-of-line blocks the stream.
            for j in range(hfree // hess_cols):
                nc.sync.dma_start(
                    out=sqd[:, j * hess_cols : (j + 1) * hess_cols], in_=hess[:, :]
                )

            # loss / d_loss pipeline. Loads on the gpsimd SWDGE ring and
            # stores on the scalar HWDGE ring so the three DMA streams
            # dispatch independently. DVE computes in place:
            #   xt: x -> d -> 2d (dloss),  yt: y -> d^2 (loss)
            for c in range(free // chunk_cols):
                sl = bass.ts(c, chunk_cols)
                xt = io.tile([P, chunk_cols], F32, tag="xt")
                yt = io.tile([P, chunk_cols], F32, tag="yt")
                nc.sync.dma_start(out=xt[:, :], in_=x[:, sl])
                nc.sync.dma_start(out=yt[:, :], in_=y[:, sl])
                d = io.tile([P, chunk_cols], F32, tag="d")
                nc.vector.tensor_sub(d[:, :], xt[:, :], yt[:, :])
                lt = io.tile([P, chunk_cols], F32, tag="lt")
                dlt = io.tile([P, chunk_cols], F32, tag="dlt")
                nc.vector.tensor_mul(lt[:, :], d[:, :], d[:, :])
                nc.vector.tensor_add(dlt[:, :], d[:, :], d[:, :])
                nc.scalar.dma_start(out=loss[:, sl], in_=lt[:, :])
                nc.scalar.dma_start(out=dloss[:, sl], in_=dlt[:, :])
    nc.finalize()
    return nc


TRACE = False  # test.py sets True to capture exec_time_ns
last_results = None


def kernel(model_out: np.ndarray, y_true: np.ndarray):
    global last_results
    model_out = np.ascontiguousarray(model_out, dtype=np.float32)
    y_true = np.ascontiguousarray(y_true, dtype=np.float32)
    free = B_SHARD * D // P

    nc = build_nc()
    in_maps = []
    for c in range(N_CORES):
        rows = slice(c * B_SHARD, (c + 1) * B_SHARD)
        in_maps.append(
            {
                "x": model_out[rows].reshape(P, free),
                "y": y_true[rows].reshape(P, free),
            }
        )
    res = run_bass_kernel_spmd(nc, in_maps, core_ids=list(range(N_CORES)), trace=TRACE)
    last_results = res

    loss = np.concatenate([r["loss"].reshape(B_SHARD, D) for r in res.results])
    dloss = np.concatenate([r["dloss"].reshape(B_SHARD, D) for r in res.results])
    sqd = np.concatenate([r["sqd"].reshape(B_SHARD, D, D) for r in res.results])
    return loss, dloss, sqd


# revision 11
# speedup vs baseline: 1.2540x; 1.0054x over previous
"""MSE AutogradLoss kernel for 8 TRN2 NeuronCores.

loss      = (x - y)^2                  [B, 16]
d_loss    = 2 (x - y)                  [B, 16]
sqd_loss  = per-sample Hessian = 2*I   [B, 16, 16]  (input-independent constant)

Pure data parallel: batch 1048576 split 8 ways (131072 rows/core).
Per core the kernel is HBM-bound: 16 MiB reads + 144 MiB writes.
The Hessian output is generated on-chip (memset pattern in SBUF) and
streamed to DRAM with large repeated DMAs - no HBM reads for it.
"""

import numpy as np

import concourse.bacc as bacc
import concourse.bass as bass
import concourse.mybir as mybir
from concourse.bass_utils import run_bass_kernel_spmd
from concourse.tile import TileContext

F32 = mybir.dt.float32

B = 1048576
D = 16
N_CORES = 8
B_SHARD = B // N_CORES  # 131072
P = 128


def build_nc(b_shard: int = B_SHARD, chunk_cols: int = 2048, hess_k: int = 32):
    """Build the per-core Bass graph.

    Layouts (per core, partition-major flattening of the batch):
      x/y/loss/dloss: [128, b_shard*16/128]   partition p holds batch rows
                                              [p*b_shard/128, (p+1)*b_shard/128)
      sqd:            [128, b_shard*256/128]  same row ownership
    """
    free = b_shard * D // P
    hfree = b_shard * D * D // P
    hess_cols = hess_k * 256
    assert free % chunk_cols == 0
    assert hfree % hess_cols == 0

    nc = bacc.Bacc()
    x = nc.declare_dram_parameter("x", [P, free], F32, isOutput=False)
    y = nc.declare_dram_parameter("y", [P, free], F32, isOutput=False)
    loss = nc.declare_dram_parameter("loss", [P, free], F32, isOutput=True)
    dloss = nc.declare_dram_parameter("dloss", [P, free], F32, isOutput=True)
    sqd = nc.declare_dram_parameter("sqd", [P, hfree], F32, isOutput=True)

    with TileContext(nc) as tc:
        with (
            tc.tile_pool(name="hpool", bufs=1) as hpool,
            tc.tile_pool(name="io", bufs=3) as io,
        ):
            # Constant Hessian pattern: per partition, hess_k copies of the
            # flattened 2*eye(16) (256 elements, 2.0 at offsets 0,17,..,255).
            hess = hpool.tile([P, hess_cols], F32)
            nc.vector.memset(hess[:, :], 0.0)
            hv = hess.rearrange("p (k c) -> p k c", c=256)
            for i in range(D):
                nc.vector.memset(hv[:, :, 17 * i], 2.0)

            # loss / d_loss pipeline, emitted FIRST so its loads lead the
            # sync-ring FIFO (the whole pipeline drains in the first ~70us).
            # bufs is sized so every chunk has its own slot - no slot-
            # recycle waits ever enter the sync FIFO. Stores go on the
            # scalar HWDGE ring (they wait on DVE; keeping them off the
            # sync ring avoids head-of-line blocking the Hessian stream).
            # DVE computes in place: xt: x -> d -> 2d, yt: y -> d^2.
            nchunks = free // chunk_cols
            for c in range(nchunks):
                sl = bass.ts(c, chunk_cols)
                xt = io.tile([P, chunk_cols], F32, tag="xt", bufs=nchunks)
                yt = io.tile([P, chunk_cols], F32, tag="yt", bufs=nchunks)
                nc.sync.dma_start(out=xt[:, :], in_=x[:, sl])
                nc.sync.dma_start(out=yt[:, :], in_=y[:, sl])
                nc.vector.tensor_sub(xt[:, :], xt[:, :], yt[:, :])
                nc.vector.tensor_mul(yt[:, :], xt[:, :], xt[:, :])
                nc.vector.tensor_add(xt[:, :], xt[:, :], xt[:, :])
                nc.scalar.dma_start(out=loss[:, sl], in_=yt[:, :])
                nc.scalar.dma_start(out=dloss[:, sl], in_=xt[:, :])

            # Stream the constant tile to the full Hessian output - the
            # dependency-free bulk (128 MiB/core) that forms the tail.
            for j in range(hfree // hess_cols):
                nc.sync.dma_start(
                    out=sqd[:, j * hess_cols : (j + 1) * hess_cols], in_=hess[:, :]
                )
    nc.finalize()
    return nc


TRACE = False  # test.py sets True to capture exec_time_ns
last_results = None


def kernel(model_out: np.ndarray, y_true: np.ndarray):
    global last_results
    model_out = np.ascontiguousarray(model_out, dtype=np.float32)
    y_true = np.ascontiguousarray(y_true, dtype=np.float32)
    free = B_SHARD * D // P

    nc = build_nc()
    in_maps = []
    for c in range(N_CORES):
        rows = slice(c * B_SHARD, (c + 1) * B_SHARD)
        in_maps.append(
            {
                "x": model_out[rows].reshape(P, free),
                "y": y_true[rows].reshape(P, free),
            }
        )
    res = run_bass_kernel_spmd(nc, in_maps, core_ids=list(range(N_CORES)), trace=TRACE)
    last_results = res

    loss = np.concatenate([r["loss"].reshape(B_SHARD, D) for r in res.results])
    dloss = np.concatenate([r["dloss"].reshape(B_SHARD, D) for r in res.results])
    sqd = np.concatenate([r["sqd"].reshape(B_SHARD, D, D) for r in res.results])
    return loss, dloss, sqd
